# revision 16
# baseline (speedup 1.0000x reference)
"""AdaptiveDisLoss Trainium2 kernel (8 NeuronCores, data-parallel over rows).

Math (mirrors the reference exactly):
  probs = softmax(x); p_true = probs[i, l_i]
  log_term_ij = min(-log(clip(p_true - p_ij, 1e-3, 1)), 5)
             == log(s_i) - log(max(e_li - e_ij, s_i * exp(-5)))   (clips collapse)
  per_true   == 5 (diff at the true column always hits the floor)
  row_sum_i  = sum_{j != l} log_term_ij = 81*log(s_i) - L_i - 5,
               L_i = sum_j log(max(e_li - e_ij, alpha*s_i)), alpha = exp(-5)
  contrib_i  = clip(1 - p_true, 1e-4, 1)^2 * row_sum_i
  loss_g     = min(sum_{i in g} contrib_i / (max(n_g,1)*80) * W_g, 1)

Device computes, per core, exp/log/segmented sums/clips and the three masked
partial sums of contrib (per partition). Host does index bookkeeping (counts,
selection masks, the per-row true-logit gather) and the final tiny divide/clamp.
"""

import numpy as np

try:
    import concourse  # noqa: F401
except ImportError:
    import sys

    for _p in ("/opt/trn_rl_repo", "/root/.axon_site/_ro/trn_rl_repo"):
        if _p not in sys.path:
            sys.path.insert(0, _p)

import concourse.bass as bass
import concourse.bacc as bacc
import concourse.tile as tile
from concourse.tile import add_dep_helper
from concourse import mybir
from concourse.bass_utils import run_bass_kernel_spmd

# Problem constants (hardcoded per spec).
N = 262144
C = 81
NUM_BASE = 60
NUM_CLASSES = 80
N_CORES = 8
NSH = N // N_CORES          # 32768 rows per core
T = 8                       # tiles per core
RT = NSH // (T * 128)       # rows per partition per tile = 32
NCOL = T * RT               # per-row buffer columns = 256
ALPHA = float(np.exp(-5.0))

W_NOVEL = 1.0 / 10
W_BASE = W_NOVEL / 3.0
W_NEG = 0.001

F32 = mybir.dt.float32
BF16 = mybir.dt.bfloat16
Alu = mybir.AluOpType
Act = mybir.ActivationFunctionType

_CACHE = {}


def _build_program():
    nc = bacc.Bacc()
    x_in = nc.declare_dram_parameter("x", [NSH, C], F32, isOutput=False)
    xl_in = nc.declare_dram_parameter("xl", [128, NCOL], F32, isOutput=False)
    mk_in = nc.declare_dram_parameter("mk", [128, 3 * NCOL], F32, isOutput=False)
    out_d = nc.declare_dram_parameter("out", [128, 4], F32, isOutput=True)

    # row = 2048*t + 16*p + r  <->  sbuf[p, col] with col = RT*t + r
    x_view = x_in[:].rearrange("(t p r) c -> t p r c", p=128, r=RT)

    with tile.TileContext(nc) as tc:
        with (
            tc.tile_pool(name="persist", bufs=1) as persist,
            tc.tile_pool(name="px", bufs=3) as px,
            tc.tile_pool(name="pe", bufs=3) as pe,
            tc.tile_pool(name="pb", bufs=3) as pb,
            tc.tile_pool(name="pm", bufs=2) as pm,
            tc.tile_pool(name="pv", bufs=T) as pv,
            tc.tile_pool(name="pl", bufs=3) as pl,
        ):
            xl_sb = persist.tile([128, NCOL], F32)
            mk_sb = persist.tile([128, 3 * NCOL], F32)
            nc.sync.dma_start(out=xl_sb, in_=xl_in[:])
            nc.sync.dma_start(out=mk_sb, in_=mk_in[:])

            el = persist.tile([128, NCOL], F32)      # e_true per row
            s_buf = persist.tile([128, NCOL], F32)   # softmax denom per row
            L_buf = persist.tile([128, NCOL], F32)   # sum_j log(max(...)) per row
            c1 = persist.tile([128, NCOL], BF16)     # e_true - alpha*s per row

            nc.scalar.activation(el, xl_sb, Act.Exp)
            el_bf = persist.tile([128, NCOL], BF16)
            nc.vector.tensor_copy(el_bf, el)

            vts = []
            last_p1_act = None
            # ---- phase 1: exp / clip (ACT: Exp + bcast Copy; GpSimd: sub) ----
            K_GPS = 0  # Pool tensor_tensor lacks min; keep min on DVE
            for t in range(T):
                cols = slice(RT * t, RT * (t + 1))
                xt = px.tile([128, RT, C], F32)
                nc.sync.dma_start(out=xt, in_=x_view[t])

                et = pe.tile([128, RT, C], BF16)
                nc.scalar.activation(et, xt, Act.Exp)

                # s = segmented row sum of e
                nc.vector.tensor_reduce(
                    s_buf[:, cols], et, axis=mybir.AxisListType.X, op=Alu.add
                )
                # c1 = e_l - alpha*s  (one fused op: (s * -alpha) + e_l)
                nc.vector.scalar_tensor_tensor(
                    out=c1[:, cols],
                    in0=s_buf[:, cols],
                    scalar=-ALPHA,
                    in1=el[:, cols],
                    op0=Alu.mult,
                    op1=Alu.add,
                )
                mt = pm.tile([128, RT, C], BF16)
                if t >= T - K_GPS:
                    # m = min(e, c1) on GpSimd (direct step-0 broadcast)
                    nc.gpsimd.tensor_tensor(
                        out=mt,
                        in0=et,
                        in1=c1[:, cols].to_broadcast([128, RT, C]),
                        op=Alu.min,
                    )
                else:
                    # broadcast-materialize c1 along the class axis on ACT so
                    # the DVE min runs in 2x flat mode
                    c1b = pb.tile([128, RT, C], BF16, tag="c1b")
                    last_p1_act = nc.scalar.activation(
                        c1b, c1[:, cols].to_broadcast([128, RT, C]), Act.Copy
                    )
                    nc.vector.tensor_tensor(
                        out=mt.rearrange("p r c -> p (r c)"),
                        in0=et.rearrange("p r c -> p (r c)"),
                        in1=c1b.rearrange("p r c -> p (r c)"),
                        op=Alu.min,
                    )
                # vneg = m - e_l  (GpSimd, step-0 broadcast in1)
                vt = pv.tile([128, RT, C], BF16)
                nc.gpsimd.tensor_tensor(
                    out=vt,
                    in0=mt,
                    in1=el_bf[:, cols].to_broadcast([128, RT, C]),
                    op=Alu.subtract,
                )
                vts.append(vt)

            # ---- phase 2: log / row sums (ACT does only Ln here) ----
            for t in range(T):
                cols = slice(RT * t, RT * (t + 1))
                lt = pl.tile([128, RT, C], BF16)
                ln_inst = nc.scalar.activation(lt, vts[t], Act.Ln, scale=-1.0)
                if t == 0 and last_p1_act is not None:
                    add_dep_helper(
                        ln_inst.ins, last_p1_act.ins, sync=False, reason="phase order"
                    )
                nc.vector.tensor_reduce(
                    L_buf[:, cols], lt, axis=mybir.AxisListType.X, op=Alu.add
                )

            # ---- per-row epilogue on [128, NCOL] ----
            logs = persist.tile([128, NCOL], F32)
            nc.scalar.activation(logs, s_buf, Act.Ln)

            rs = persist.tile([128, NCOL], F32)
            # rs = 81*log(s) - L
            nc.vector.scalar_tensor_tensor(
                out=rs, in0=logs, scalar=float(C), in1=L_buf,
                op0=Alu.mult, op1=Alu.subtract,
            )
            # rs2 = rs - 5
            rs2 = persist.tile([128, NCOL], F32)
            nc.vector.tensor_scalar(rs2, rs, -5.0, None, Alu.add)

            rinv = persist.tile([128, NCOL], F32)
            nc.vector.reciprocal(rinv, s_buf)
            pt = persist.tile([128, NCOL], F32)
            nc.vector.tensor_tensor(out=pt, in0=el, in1=rinv, op=Alu.mult)

            # omp = clip(1 - p_true, 1e-4, 1)
            omp = persist.tile([128, NCOL], F32)
            nc.vector.tensor_scalar(omp, pt, -1.0, 1.0, Alu.mult, Alu.add)
            ompc = persist.tile([128, NCOL], F32)
            nc.vector.tensor_scalar(ompc, omp, 1e-4, 1.0, Alu.max, Alu.min)

            w = persist.tile([128, NCOL], F32)
            nc.scalar.activation(w, ompc, Act.Square)
            contrib = persist.tile([128, NCOL], F32)
            nc.vector.tensor_tensor(out=contrib, in0=w, in1=rs2, op=Alu.mult)

            osb = persist.tile([128, 4], F32)
            nc.vector.memset(osb, 0.0)
            scr = persist.tile([128, NCOL], F32)
            for g in range(3):
                nc.vector.tensor_tensor(
                    out=scr,
                    in0=contrib,
                    in1=mk_sb[:, g * NCOL : (g + 1) * NCOL],
                    op=Alu.mult,
                )
                nc.vector.tensor_reduce(
                    osb[:, g : g + 1], scr, axis=mybir.AxisListType.X, op=Alu.add
                )
            nc.sync.dma_start(out=out_d[:], in_=osb)

    nc.finalize()
    return nc


def _get_program():
    if "nc" not in _CACHE:
        _CACHE["nc"] = _build_program()
    return _CACHE["nc"]


def _row_layout(a):
    """[NSH] per-core array -> [128, NCOL] with col = RT*t + r, row = 2048t+16p+r."""
    return a.reshape(T, 128, RT).transpose(1, 0, 2).reshape(128, NCOL)


def prepare_inputs(cls_score, labels, label_weights):
    x = np.ascontiguousarray(np.asarray(cls_score, dtype=np.float32))
    lab = np.asarray(labels).astype(np.int64)
    lw = np.asarray(label_weights, dtype=np.float32)

    valid = lw > 0
    counts = np.bincount(lab[valid], minlength=C)
    enough = counts[lab] >= 2
    base_sel = valid & (lab < NUM_BASE) & enough
    novel_sel = valid & (lab >= NUM_BASE) & (lab < NUM_CLASSES) & enough
    neg_sel = valid & (lab == NUM_CLASSES)

    xl = np.take_along_axis(x, lab[:, None].astype(np.int64), axis=1)[:, 0]
    masks = np.stack(
        [base_sel.astype(np.float32), novel_sel.astype(np.float32),
         neg_sel.astype(np.float32)]
    )  # [3, N]

    in_maps = []
    for i in range(N_CORES):
        sl = slice(i * NSH, (i + 1) * NSH)
        mk = np.concatenate(
            [_row_layout(masks[g, sl]) for g in range(3)], axis=1
        )  # [128, 3*NCOL]
        in_maps.append(
            {
                "x": np.ascontiguousarray(x[sl]),
                "xl": np.ascontiguousarray(_row_layout(xl[sl])),
                "mk": np.ascontiguousarray(mk),
            }
        )
    ns = (int(base_sel.sum()), int(novel_sel.sum()), int(neg_sel.sum()))
    return in_maps, ns


def finalize(results, ns):
    sums = np.zeros(3, dtype=np.float64)
    for r in results:
        o = np.asarray(r["out"], dtype=np.float64)
        sums += o[:, :3].sum(axis=0)
    losses = []
    for g, wg in enumerate((W_BASE, W_NOVEL, W_NEG)):
        n = ns[g]
        if n > 0:
            mean = sums[g] / (max(n, 1) * (C - 1))
        else:
            mean = 0.0
        losses.append(np.float32(min(mean * wg, 1.0)))
    return tuple(losses)


def kernel(cls_score, labels, label_weights, _trace=False, _tmpdir=None):
    nc = _get_program()
    in_maps, ns = prepare_inputs(cls_score, labels, label_weights)
    res = run_bass_kernel_spmd(
        nc, in_maps, core_ids=list(range(N_CORES)), trace=_trace, tmpdir=_tmpdir
    )
    out = finalize(res.results, ns)
    if _trace:
        return out, res
    return out


# revision 17
# speedup vs baseline: 1.1348x; 1.1348x over previous
"""AdaptiveDisLoss Trainium2 kernel (8 NeuronCores, data-parallel over rows).

Math (mirrors the reference exactly):
  probs = softmax(x); p_true = probs[i, l_i]
  log_term_ij = min(-log(clip(p_true - p_ij, 1e-3, 1)), 5)
             == log(s_i) - log(max(e_li - e_ij, s_i * exp(-5)))   (clips collapse)
  per_true   == 5 (diff at the true column always hits the floor)
  row_sum_i  = sum_{j != l} log_term_ij = 81*log(s_i) - L_i - 5,
               L_i = sum_j log(max(e_li - e_ij, alpha*s_i)), alpha = exp(-5)
  contrib_i  = clip(1 - p_true, 1e-4, 1)^2 * row_sum_i
  loss_g     = min(sum_{i in g} contrib_i / (max(n_g,1)*80) * W_g, 1)

Device computes, per core, exp/log/segmented sums/clips and the three masked
partial sums of contrib (per partition). Host does index bookkeeping (counts,
selection masks, the per-row true-logit gather) and the final tiny divide/clamp.
"""

import numpy as np

try:
    import concourse  # noqa: F401
except ImportError:
    import sys

    for _p in ("/opt/trn_rl_repo", "/root/.axon_site/_ro/trn_rl_repo"):
        if _p not in sys.path:
            sys.path.insert(0, _p)

import concourse.bass as bass
import concourse.bacc as bacc
import concourse.tile as tile
from concourse.tile import add_dep_helper
from concourse import mybir
from concourse.bass_utils import run_bass_kernel_spmd

# Problem constants (hardcoded per spec).
N = 262144
C = 81
NUM_BASE = 60
NUM_CLASSES = 80
N_CORES = 8
NSH = N // N_CORES          # 32768 rows per core
T = 8                       # tiles per core
RT = NSH // (T * 128)       # rows per partition per tile = 32
NCOL = T * RT               # per-row buffer columns = 256
ALPHA = float(np.exp(-5.0))

W_NOVEL = 1.0 / 10
W_BASE = W_NOVEL / 3.0
W_NEG = 0.001

F32 = mybir.dt.float32
BF16 = mybir.dt.bfloat16
Alu = mybir.AluOpType
Act = mybir.ActivationFunctionType

_CACHE = {}


def _build_program():
    nc = bacc.Bacc()
    x_in = nc.declare_dram_parameter("x", [NSH, C], F32, isOutput=False)
    xl_in = nc.declare_dram_parameter("xl", [128, NCOL], F32, isOutput=False)
    mk_in = nc.declare_dram_parameter("mk", [128, 3 * NCOL], F32, isOutput=False)
    out_d = nc.declare_dram_parameter("out", [128, 4], F32, isOutput=True)

    # row = 2048*t + 16*p + r  <->  sbuf[p, col] with col = RT*t + r
    x_view = x_in[:].rearrange("(t p r) c -> t p r c", p=128, r=RT)

    with tile.TileContext(nc) as tc:
        with (
            tc.tile_pool(name="persist", bufs=1) as persist,
            tc.tile_pool(name="px", bufs=3) as px,
            tc.tile_pool(name="pe", bufs=3) as pe,
            tc.tile_pool(name="pb", bufs=3) as pb,
            tc.tile_pool(name="pm", bufs=2) as pm,
            tc.tile_pool(name="pv", bufs=T) as pv,
            tc.tile_pool(name="pl", bufs=3) as pl,
        ):
            xl_sb = persist.tile([128, NCOL], F32)
            mk_sb = persist.tile([128, 3 * NCOL], F32)
            nc.sync.dma_start(out=xl_sb, in_=xl_in[:])
            nc.sync.dma_start(out=mk_sb, in_=mk_in[:])

            el = persist.tile([128, NCOL], F32)      # e_true per row
            s_buf = persist.tile([128, NCOL], F32)   # softmax denom per row
            L_buf = persist.tile([128, NCOL], F32)   # sum_j log(max(...)) per row
            c1 = persist.tile([128, NCOL], BF16)     # e_true - alpha*s per row

            nc.scalar.activation(el, xl_sb, Act.Exp)
            el_bf = persist.tile([128, NCOL], BF16)
            nc.vector.tensor_copy(el_bf, el)

            vts = []
            last_p1_act = None
            # ---- phase 1: exp / clip (ACT: Exp + bcast Copy; GpSimd: sub) ----
            K_GPS = 0  # Pool tensor_tensor lacks min; keep min on DVE
            for t in range(T):
                cols = slice(RT * t, RT * (t + 1))
                xt = px.tile([128, RT, C], F32)
                nc.gpsimd.dma_start(out=xt, in_=x_view[t])

                et = pe.tile([128, RT, C], BF16)
                nc.scalar.activation(et, xt, Act.Exp)

                # s = segmented row sum of e
                nc.vector.tensor_reduce(
                    s_buf[:, cols], et, axis=mybir.AxisListType.X, op=Alu.add
                )
                # c1 = e_l - alpha*s  (one fused op: (s * -alpha) + e_l)
                nc.vector.scalar_tensor_tensor(
                    out=c1[:, cols],
                    in0=s_buf[:, cols],
                    scalar=-ALPHA,
                    in1=el[:, cols],
                    op0=Alu.mult,
                    op1=Alu.add,
                )
                mt = pm.tile([128, RT, C], BF16)
                if t >= T - K_GPS:
                    # m = min(e, c1) on GpSimd (direct step-0 broadcast)
                    nc.gpsimd.tensor_tensor(
                        out=mt,
                        in0=et,
                        in1=c1[:, cols].to_broadcast([128, RT, C]),
                        op=Alu.min,
                    )
                else:
                    # broadcast-materialize c1 along the class axis on ACT so
                    # the DVE min runs in 2x flat mode
                    c1b = pb.tile([128, RT, C], BF16, tag="c1b")
                    last_p1_act = nc.scalar.activation(
                        c1b, c1[:, cols].to_broadcast([128, RT, C]), Act.Copy
                    )
                    nc.vector.tensor_tensor(
                        out=mt.rearrange("p r c -> p (r c)"),
                        in0=et.rearrange("p r c -> p (r c)"),
                        in1=c1b.rearrange("p r c -> p (r c)"),
                        op=Alu.min,
                    )
                # vneg = m - e_l  (GpSimd, step-0 broadcast in1)
                vt = pv.tile([128, RT, C], BF16)
                nc.gpsimd.tensor_tensor(
                    out=vt,
                    in0=mt,
                    in1=el_bf[:, cols].to_broadcast([128, RT, C]),
                    op=Alu.subtract,
                )
                vts.append(vt)

            # ---- phase 2: log / row sums (ACT does only Ln here) ----
            for t in range(T):
                cols = slice(RT * t, RT * (t + 1))
                lt = pl.tile([128, RT, C], BF16)
                ln_inst = nc.scalar.activation(lt, vts[t], Act.Ln, scale=-1.0)
                if t == 0 and last_p1_act is not None:
                    add_dep_helper(
                        ln_inst.ins, last_p1_act.ins, sync=False, reason="phase order"
                    )
                nc.vector.tensor_reduce(
                    L_buf[:, cols], lt, axis=mybir.AxisListType.X, op=Alu.add
                )

            # ---- per-row epilogue on [128, NCOL] ----
            logs = persist.tile([128, NCOL], F32)
            nc.scalar.activation(logs, s_buf, Act.Ln)

            rs = persist.tile([128, NCOL], F32)
            # rs = 81*log(s) - L
            nc.vector.scalar_tensor_tensor(
                out=rs, in0=logs, scalar=float(C), in1=L_buf,
                op0=Alu.mult, op1=Alu.subtract,
            )
            # rs2 = rs - 5
            rs2 = persist.tile([128, NCOL], F32)
            nc.vector.tensor_scalar(rs2, rs, -5.0, None, Alu.add)

            rinv = persist.tile([128, NCOL], F32)
            nc.vector.reciprocal(rinv, s_buf)
            pt = persist.tile([128, NCOL], F32)
            nc.vector.tensor_tensor(out=pt, in0=el, in1=rinv, op=Alu.mult)

            # omp = clip(1 - p_true, 1e-4, 1)
            omp = persist.tile([128, NCOL], F32)
            nc.vector.tensor_scalar(omp, pt, -1.0, 1.0, Alu.mult, Alu.add)
            ompc = persist.tile([128, NCOL], F32)
            nc.vector.tensor_scalar(ompc, omp, 1e-4, 1.0, Alu.max, Alu.min)

            w = persist.tile([128, NCOL], F32)
            nc.scalar.activation(w, ompc, Act.Square)
            contrib = persist.tile([128, NCOL], F32)
            nc.vector.tensor_tensor(out=contrib, in0=w, in1=rs2, op=Alu.mult)

            osb = persist.tile([128, 4], F32)
            nc.vector.memset(osb, 0.0)
            scr = persist.tile([128, NCOL], F32)
            for g in range(3):
                nc.vector.tensor_tensor(
                    out=scr,
                    in0=contrib,
                    in1=mk_sb[:, g * NCOL : (g + 1) * NCOL],
                    op=Alu.mult,
                )
                nc.vector.tensor_reduce(
                    osb[:, g : g + 1], scr, axis=mybir.AxisListType.X, op=Alu.add
                )
            nc.sync.dma_start(out=out_d[:], in_=osb)

    nc.finalize()
    return nc


def _get_program():
    if "nc" not in _CACHE:
        _CACHE["nc"] = _build_program()
    return _CACHE["nc"]


def _row_layout(a):
    """[NSH] per-core array -> [128, NCOL] with col = RT*t + r, row = 2048t+16p+r."""
    return a.reshape(T, 128, RT).transpose(1, 0, 2).reshape(128, NCOL)


def prepare_inputs(cls_score, labels, label_weights):
    x = np.ascontiguousarray(np.asarray(cls_score, dtype=np.float32))
    lab = np.asarray(labels).astype(np.int64)
    lw = np.asarray(label_weights, dtype=np.float32)

    valid = lw > 0
    counts = np.bincount(lab[valid], minlength=C)
    enough = counts[lab] >= 2
    base_sel = valid & (lab < NUM_BASE) & enough
    novel_sel = valid & (lab >= NUM_BASE) & (lab < NUM_CLASSES) & enough
    neg_sel = valid & (lab == NUM_CLASSES)

    xl = np.take_along_axis(x, lab[:, None].astype(np.int64), axis=1)[:, 0]
    masks = np.stack(
        [base_sel.astype(np.float32), novel_sel.astype(np.float32),
         neg_sel.astype(np.float32)]
    )  # [3, N]

    in_maps = []
    for i in range(N_CORES):
        sl = slice(i * NSH, (i + 1) * NSH)
        mk = np.concatenate(
            [_row_layout(masks[g, sl]) for g in range(3)], axis=1
        )  # [128, 3*NCOL]
        in_maps.append(
            {
                "x": np.ascontiguousarray(x[sl]),
                "xl": np.ascontiguousarray(_row_layout(xl[sl])),
                "mk": np.ascontiguousarray(mk),
            }
        )
    ns = (int(base_sel.sum()), int(novel_sel.sum()), int(neg_sel.sum()))
    return in_maps, ns


def finalize(results, ns):
    sums = np.zeros(3, dtype=np.float64)
    for r in results:
        o = np.asarray(r["out"], dtype=np.float64)
        sums += o[:, :3].sum(axis=0)
    losses = []
    for g, wg in enumerate((W_BASE, W_NOVEL, W_NEG)):
        n = ns[g]
        if n > 0:
            mean = sums[g] / (max(n, 1) * (C - 1))
        else:
            mean = 0.0
        losses.append(np.float32(min(mean * wg, 1.0)))
    return tuple(losses)


def kernel(cls_score, labels, label_weights, _trace=False, _tmpdir=None):
    nc = _get_program()
    in_maps, ns = prepare_inputs(cls_score, labels, label_weights)
    res = run_bass_kernel_spmd(
        nc, in_maps, core_ids=list(range(N_CORES)), trace=_trace, tmpdir=_tmpdir
    )
    out = finalize(res.results, ns)
    if _trace:
        return out, res
    return out
